# revision 19
# baseline (speedup 1.0000x reference)
"""DeepseekV3 MoE experts kernel for 8 Trainium2 NeuronCores.

Problem: every expert processes the FULL token set.
  g = x @ w_gate[e].T ; u = x @ w_up[e].T ; h = silu(g)*u
  out[e] = h @ w_down[e].T ;  concat over e -> [E*T, H]

Sharding: expert-parallel. Core c owns experts {2c, 2c+1}; hidden_states is
replicated; outputs are concatenated host-side (no on-device collectives).

Per-core compute (per expert e, with xT = x.T resident in SBUF):
  phase 1: gT[m*128:(m+1)*128, :] = wgT[:,k,mslice].T @ xT[:,k,:]  (acc over k)
  phase 2: same for uT; hT = silu(gT) * uT  (in [I, T] layout, no transposes)
  phase 3: out[mt*128.., nslice] = hT[:,k,mtslice].T @ wdT[:,k,nslice]

All matmul operands are bf16 (host-cast); PSUM accumulation is fp32 and the
output is written back in fp32. Weights stream in eighth-matrix tiles
(0.5 MiB DMAs) so the PE starts as soon as the first slice lands; the slot
pools back-pressure the DMA stream about one expert ahead.

Self-contained: shapes hardcoded; inputs are the full arrays from
setup_inputs(); returns the full [4096, 2048] fp32 output.
"""

import numpy as np
import ml_dtypes

E, T, H, I = 16, 256, 2048, 1024
N_CORES = 8
E_PER = E // N_CORES  # 2
P = 128
KO = H // P  # 16 k-chunks for phases 1/2
MO = I // P  # 8 m-chunks for phases 1/2 (= k-chunks for phase 3)
TO = T // P  # 2 m-chunks for phase 3
NS = 512  # n-slice width for phase 3
NH = H // NS  # 4

KQ = 2  # k-chunks per wg/wu eighth-tile (0.5 MiB DMAs: earlier PE start)
DQ = 2  # k-chunks per wd quarter-tile
XH = KO // 2  # k-chunks per x half-tile

_CACHE: dict = {}


def _build_program(sim_compat=False):
    # sim_compat: CoreSim lacks the Silu LUT — express silu as
    # sigmoid(g)*g with an extra DVE multiply. HW uses the fused Silu op.
    import concourse.mybir as mybir
    import concourse.tile as tile
    from concourse import bacc

    dt = mybir.dt.bfloat16
    f32 = mybir.dt.float32
    AF = mybir.ActivationFunctionType

    nc = bacc.Bacc(None, target_bir_lowering=False, debug=False)

    xT = nc.dram_tensor("xT", [P, KO, T], dt, kind="ExternalInput")[:]
    wg = nc.dram_tensor("wg", [E_PER, P, KO, I], dt, kind="ExternalInput")[:]
    wu = nc.dram_tensor("wu", [E_PER, P, KO, I], dt, kind="ExternalInput")[:]
    wd = nc.dram_tensor("wd", [E_PER, P, MO, H], dt, kind="ExternalInput")[:]
    # output in bf16: halves the output DMA bytes and the PSUM->SBUF copy
    # time; the host upcasts to fp32 (rounding adds ~1e-3 rel err, well
    # within the 2e-2 budget)
    out = nc.dram_tensor("out", [E_PER, TO, P, H], dt, kind="ExternalOutput")[:]

    with tile.TileContext(nc) as tc:
        with (
            tc.tile_pool(name="xp", bufs=6) as xp,
            tc.tile_pool(name="wp", bufs=26) as wp,
            tc.tile_pool(name="wdp", bufs=6) as wdp,
            tc.tile_pool(name="hp", bufs=2) as hp,
            tc.tile_pool(name="gp", bufs=8) as gp,
            tc.tile_pool(name="op", bufs=8) as outp,
            tc.tile_pool(name="ps", bufs=8, space="PSUM") as ps,
        ):
            # PE warm-up: matmuls on an uninitialized scratch tile with no
            # producer dependency, so they issue the moment the boot
            # barrier releases (~6.6us) and run while the first input DMAs
            # are still in flight. This flips the HAM clock gate to 8/8
            # before the first real matmul. 18 cold MMs ~= 4.4us of
            # sustained PE activity; the first real weight tiles land at
            # ~10.5us, right as the warm-up drains. (Values are garbage;
            # the PSUM tile is never read.)
            warm = xp.tile([P, T], dt, tag="warm")
            nc.vector.memset(warm[:], 0.0)
            wps = ps.tile([P, T], f32, tag="ps")
            for _ in range(18):
                nc.tensor.matmul(wps[:], warm[:, :P], warm[:], start=True, stop=True)

            xtiles = []  # (k0, kq, tile)
            wq: dict = {}  # (e, name) -> list of (k0, kq, tile)

            def issue_x(k0, kq, eng=None):
                t_ = xp.tile([P, kq, T], dt, tag="x")
                (eng or nc.sync).dma_start(t_[:], xT[:, k0 : k0 + kq, :])
                xtiles.append((k0, kq, t_))

            def xslice(k):
                for k0, kq, t_ in xtiles:
                    if k0 <= k < k0 + kq:
                        return t_[:, k - k0, :]
                raise KeyError(k)

            def issue_w(e, name, k0, kq, src, pool, eng=None):
                t_ = pool.tile([P, kq, src.shape[3]], dt, tag=pool.name)
                (eng or nc.sync).dma_start(t_[:], src[e, :, k0 : k0 + kq, :])
                wq.setdefault((e, name), []).append((k0, kq, t_))

            def wslice(e, name, k, lo, hi):
                for k0, kq, t_ in wq[(e, name)]:
                    if k0 <= k < k0 + kq:
                        return t_[:, k - k0, lo:hi]
                raise KeyError((e, name, k))

            # DMA issue order == consumption order. The leading slices are
            # extra small so the first matmul starts ~3us earlier; x is
            # woven into the first gate stream so the k-loop is never
            # input-starved. The slot pools back-pressure the stream.
            # All DMAs go through the SP HWDGE ring (nc.sync) — issuing
            # from the ACT ring measures consistently slower here. The
            # weave matches the gate k-outer consumption order: each x
            # chunk lands just before the wg chunks of the same k range.
            # Leading wg/wu chunks go out as 0.25 MiB kq=1 transfers:
            # during DMA spin-up each lane's first transfer pays ~2us of
            # completion latency, so more, smaller transfers engage all
            # lanes sooner and cut the arrival quantization the gate
            # k-loop sees.
            issue_x(0, 2)
            issue_w(0, "wg", 0, 1, wg, wp)
            issue_w(0, "wg", 1, 1, wg, wp)
            issue_x(2, 2)
            issue_w(0, "wg", 2, 1, wg, wp)
            issue_w(0, "wg", 3, 1, wg, wp)
            issue_x(4, 4)
            issue_w(0, "wg", 4, 1, wg, wp)
            issue_w(0, "wg", 5, 1, wg, wp)
            issue_w(0, "wg", 6, 1, wg, wp)
            issue_w(0, "wg", 7, 1, wg, wp)
            issue_x(8, 4)
            issue_w(0, "wg", 8, 2, wg, wp)
            issue_w(0, "wg", 10, 2, wg, wp)
            issue_x(12, 4)
            for k0 in range(12, KO, KQ):
                issue_w(0, "wg", k0, KQ, wg, wp)
            for k0 in range(0, 4):
                issue_w(0, "wu", k0, 1, wu, wp)
            for k0 in range(4, KO, KQ):
                issue_w(0, "wu", k0, KQ, wu, wp)
            for k0 in range(0, MO, DQ):
                issue_w(0, "wd", k0, DQ, wd, wdp)
            for e in range(1, E_PER):
                for k0 in range(0, KO, KQ):
                    issue_w(e, "wg", k0, KQ, wg, wp)
                for k0 in range(0, KO, KQ):
                    issue_w(e, "wu", k0, KQ, wu, wp)
                for k0 in range(0, MO, DQ):
                    issue_w(e, "wd", k0, DQ, wd, wdp)

            for e in range(E_PER):
                hT = hp.tile([P, MO, T], dt, tag="h")
                # all gate m-chunks first: the PE is gated only by the wg
                # stream, never by wu (which lands while these run).
                # Expert 0's gate runs k-outer so each wg tile is consumed
                # the moment its DMA lands (the stream is still ramping);
                # by expert 1 the weights are fully buffered and m-outer
                # staggers the silu/mult chain better.
                gss = []
                if e == 0:
                    pgs = [
                        ps.tile([P, T], f32, name=f"pg{m}", tag="ps")
                        for m in range(MO)
                    ]
                    for k in range(KO):
                        for m in range(MO):
                            nc.tensor.matmul(
                                pgs[m][:],
                                wslice(e, "wg", k, m * P, (m + 1) * P),
                                xslice(k),
                                start=(k == 0),
                                stop=(k == KO - 1),
                            )
                    for m in range(MO):
                        gs = gp.tile([P, T], f32, tag="g")
                        if sim_compat:
                            nc.scalar.activation(gs[:], pgs[m][:], AF.Sigmoid)
                            nc.vector.tensor_tensor(
                                gs[:], gs[:], pgs[m][:], mybir.AluOpType.mult
                            )
                        else:
                            nc.scalar.activation(gs[:], pgs[m][:], AF.Silu)
                        gss.append(gs)
                else:
                    for m in range(MO):
                        pg = ps.tile([P, T], f32, tag="ps")
                        for k in range(KO):
                            nc.tensor.matmul(
                                pg[:],
                                wslice(e, "wg", k, m * P, (m + 1) * P),
                                xslice(k),
                                start=(k == 0),
                                stop=(k == KO - 1),
                            )
                        gs = gp.tile([P, T], f32, tag="g")
                        if sim_compat:
                            nc.scalar.activation(gs[:], pg[:], AF.Sigmoid)
                            nc.vector.tensor_tensor(
                                gs[:], gs[:], pg[:], mybir.AluOpType.mult
                            )
                        else:
                            nc.scalar.activation(gs[:], pg[:], AF.Silu)
                        gss.append(gs)
                for m in range(MO):
                    pu = ps.tile([P, T], f32, tag="ps")
                    for k in range(KO):
                        nc.tensor.matmul(
                            pu[:],
                            wslice(e, "wu", k, m * P, (m + 1) * P),
                            xslice(k),
                            start=(k == 0),
                            stop=(k == KO - 1),
                        )
                    nc.vector.tensor_tensor(
                        hT[:, m, :], gss[m][:], pu[:], mybir.AluOpType.mult
                    )
                for mt in range(TO):
                    for n in range(NH):
                        po = ps.tile([P, NS], f32, tag="ps")
                        for k in range(MO):
                            nc.tensor.matmul(
                                po[:],
                                hT[:, k, mt * P : (mt + 1) * P],
                                wslice(e, "wd", k, n * NS, (n + 1) * NS),
                                start=(k == 0),
                                stop=(k == MO - 1),
                            )
                        # The very last tile is split in half so copy2
                        # overlaps dma1, shortening the end-of-kernel
                        # chain.
                        last = e == E_PER - 1 and mt == TO - 1 and n == NH - 1
                        if last:
                            oa = outp.tile([P, NS // 2], dt, tag="o")
                            nc.scalar.copy(oa[:], po[:, : NS // 2])
                            nc.sync.dma_start(
                                out[e, mt, :, n * NS : n * NS + NS // 2], oa[:]
                            )
                            ob = outp.tile([P, NS // 2], dt, tag="o")
                            nc.scalar.copy(ob[:], po[:, NS // 2 :])
                            nc.sync.dma_start(
                                out[e, mt, :, n * NS + NS // 2 : (n + 1) * NS], ob[:]
                            )
                        else:
                            ot = outp.tile([P, NS], dt, tag="o")
                            nc.scalar.copy(ot[:], po[:])
                            nc.sync.dma_start(
                                out[e, mt, :, n * NS : (n + 1) * NS], ot[:]
                            )

    nc.compile()
    return nc


def get_program(sim_compat=False):
    key = ("nc", sim_compat)
    if key not in _CACHE:
        _CACHE[key] = _build_program(sim_compat=sim_compat)
    return _CACHE[key]


def _prep_in_maps(hidden_states, w_gate, w_up, w_down):
    bf16 = ml_dtypes.bfloat16
    x = np.asarray(hidden_states, dtype=np.float32)
    wg = np.asarray(w_gate, dtype=np.float32)
    wu = np.asarray(w_up, dtype=np.float32)
    wd = np.asarray(w_down, dtype=np.float32)

    # xT: [H, T] -> [128, KO, T], partition p + chunk k <-> H index k*128+p
    xt = np.ascontiguousarray(
        x.T.reshape(KO, P, T).transpose(1, 0, 2).astype(bf16)
    )
    # w_gate/w_up: [E, I, H] -> per expert W.T = [H, I] -> [128, KO, I]
    wgt = np.ascontiguousarray(
        wg.transpose(0, 2, 1).reshape(E, KO, P, I).transpose(0, 2, 1, 3).astype(bf16)
    )
    wut = np.ascontiguousarray(
        wu.transpose(0, 2, 1).reshape(E, KO, P, I).transpose(0, 2, 1, 3).astype(bf16)
    )
    # w_down: [E, H, I] -> per expert W.T = [I, H] -> [128, MO, H]
    wdt = np.ascontiguousarray(
        wd.transpose(0, 2, 1).reshape(E, MO, P, H).transpose(0, 2, 1, 3).astype(bf16)
    )

    in_maps = []
    for c in range(N_CORES):
        sl = slice(c * E_PER, (c + 1) * E_PER)
        in_maps.append(
            {
                "xT": xt,
                "wg": np.ascontiguousarray(wgt[sl]),
                "wu": np.ascontiguousarray(wut[sl]),
                "wd": np.ascontiguousarray(wdt[sl]),
            }
        )
    return in_maps


def kernel(hidden_states, w_gate, w_up, w_down, _trace=False, _trace_kwargs=None):
    from concourse.bass_utils import run_bass_kernel_spmd

    nc = get_program()
    in_maps = _prep_in_maps(hidden_states, w_gate, w_up, w_down)
    kwargs = {}
    if _trace:
        kwargs = dict(trace=True, **(_trace_kwargs or {}))
    res = run_bass_kernel_spmd(nc, in_maps, core_ids=list(range(N_CORES)), **kwargs)
    out = np.concatenate(
        [
            res.results[c]["out"].reshape(E_PER * T, H).astype(np.float32)
            for c in range(N_CORES)
        ],
        axis=0,
    )
    if _trace:
        _CACHE["last_results"] = res
    return out



# revision 22
# speedup vs baseline: 1.0047x; 1.0047x over previous
"""DeepseekV3 MoE experts kernel for 8 Trainium2 NeuronCores.

Problem: every expert processes the FULL token set.
  g = x @ w_gate[e].T ; u = x @ w_up[e].T ; h = silu(g)*u
  out[e] = h @ w_down[e].T ;  concat over e -> [E*T, H]

Sharding: expert-parallel. Core c owns experts {2c, 2c+1}; hidden_states is
replicated; outputs are concatenated host-side (no on-device collectives).

Per-core compute (per expert e, with xT = x.T resident in SBUF):
  phase 1: gT[m*128:(m+1)*128, :] = wgT[:,k,mslice].T @ xT[:,k,:]  (acc over k)
  phase 2: same for uT; hT = silu(gT) * uT  (in [I, T] layout, no transposes)
  phase 3: out[mt*128.., nslice] = hT[:,k,mtslice].T @ wdT[:,k,nslice]

All matmul operands are bf16 (host-cast); PSUM accumulation is fp32 and the
output is written back in fp32. Weights stream in eighth-matrix tiles
(0.5 MiB DMAs) so the PE starts as soon as the first slice lands; the slot
pools back-pressure the DMA stream about one expert ahead.

Self-contained: shapes hardcoded; inputs are the full arrays from
setup_inputs(); returns the full [4096, 2048] fp32 output.
"""

import numpy as np
import ml_dtypes

E, T, H, I = 16, 256, 2048, 1024
N_CORES = 8
E_PER = E // N_CORES  # 2
P = 128
KO = H // P  # 16 k-chunks for phases 1/2
MO = I // P  # 8 m-chunks for phases 1/2 (= k-chunks for phase 3)
TO = T // P  # 2 m-chunks for phase 3
NS = 512  # n-slice width for phase 3
NH = H // NS  # 4

KQ = 2  # k-chunks per wg/wu eighth-tile (0.5 MiB DMAs: earlier PE start)
DQ = 2  # k-chunks per wd quarter-tile
XH = KO // 2  # k-chunks per x half-tile

_CACHE: dict = {}


def _build_program(sim_compat=False):
    # sim_compat: CoreSim lacks the Silu LUT — express silu as
    # sigmoid(g)*g with an extra DVE multiply. HW uses the fused Silu op.
    import concourse.mybir as mybir
    import concourse.tile as tile
    from concourse import bacc

    dt = mybir.dt.bfloat16
    f32 = mybir.dt.float32
    AF = mybir.ActivationFunctionType

    nc = bacc.Bacc(None, target_bir_lowering=False, debug=False)

    xT = nc.dram_tensor("xT", [P, KO, T], dt, kind="ExternalInput")[:]
    wg = nc.dram_tensor("wg", [E_PER, P, KO, I], dt, kind="ExternalInput")[:]
    wu = nc.dram_tensor("wu", [E_PER, P, KO, I], dt, kind="ExternalInput")[:]
    wd = nc.dram_tensor("wd", [E_PER, P, MO, H], dt, kind="ExternalInput")[:]
    # output in bf16: halves the output DMA bytes and the PSUM->SBUF copy
    # time; the host upcasts to fp32 (rounding adds ~1e-3 rel err, well
    # within the 2e-2 budget)
    out = nc.dram_tensor("out", [E_PER, TO, P, H], dt, kind="ExternalOutput")[:]

    with tile.TileContext(nc) as tc:
        with (
            tc.tile_pool(name="xp", bufs=6) as xp,
            tc.tile_pool(name="wp", bufs=24) as wp,
            tc.tile_pool(name="wdp", bufs=6) as wdp,
            tc.tile_pool(name="hp", bufs=2) as hp,
            tc.tile_pool(name="gp", bufs=8) as gp,
            tc.tile_pool(name="op", bufs=8) as outp,
            tc.tile_pool(name="ps", bufs=8, space="PSUM") as ps,
        ):
            # PE warm-up: matmuls on an uninitialized scratch tile with no
            # producer dependency, so they issue the moment the boot
            # barrier releases (~6.6us) and run while the first input DMAs
            # are still in flight. This flips the HAM clock gate to 8/8
            # before the first real matmul. 18 cold MMs ~= 4.4us of
            # sustained PE activity; the first real weight tiles land at
            # ~10.5us, right as the warm-up drains. (Values are garbage;
            # the PSUM tile is never read.)
            warm = xp.tile([P, T], dt, tag="warm")
            nc.vector.memset(warm[:], 0.0)
            wps = ps.tile([P, T], f32, tag="ps")
            for _ in range(16):
                nc.tensor.matmul(wps[:], warm[:, :P], warm[:], start=True, stop=True)

            xtiles = []  # (k0, kq, tile)
            wq: dict = {}  # (e, name) -> list of (k0, kq, tile)

            def issue_x(k0, kq, eng=None):
                t_ = xp.tile([P, kq, T], dt, tag="x")
                (eng or nc.sync).dma_start(t_[:], xT[:, k0 : k0 + kq, :])
                xtiles.append((k0, kq, t_))

            def xslice(k):
                for k0, kq, t_ in xtiles:
                    if k0 <= k < k0 + kq:
                        return t_[:, k - k0, :]
                raise KeyError(k)

            def issue_w(e, name, k0, kq, src, pool, eng=None):
                t_ = pool.tile([P, kq, src.shape[3]], dt, tag=pool.name)
                (eng or nc.sync).dma_start(t_[:], src[e, :, k0 : k0 + kq, :])
                wq.setdefault((e, name), []).append((k0, kq, t_))

            def wslice(e, name, k, lo, hi):
                for k0, kq, t_ in wq[(e, name)]:
                    if k0 <= k < k0 + kq:
                        return t_[:, k - k0, lo:hi]
                raise KeyError((e, name, k))

            # DMA issue order == consumption order. The leading slices are
            # extra small so the first matmul starts ~3us earlier; x is
            # woven into the first gate stream so the k-loop is never
            # input-starved. The slot pools back-pressure the stream.
            # All DMAs go through the SP HWDGE ring (nc.sync) — issuing
            # from the ACT ring measures consistently slower here. The
            # weave matches the gate k-outer consumption order: each x
            # chunk lands just before the wg chunks of the same k range.
            # x and the first two wg chunks go via SWDGE (nc.gpsimd) — a
            # separate descriptor engine that is idle at boot — so they
            # stream in parallel with the SP HWDGE ring while the DMA
            # subsystem spins up. Everything else stays on the SP ring
            # (the ACT ring measured slower; small kq=1 transfers beyond
            # the first two also measured slower).
            GPD = nc.gpsimd
            issue_x(0, 2, GPD)
            issue_w(0, "wg", 0, 1, wg, wp, GPD)
            issue_w(0, "wg", 1, 1, wg, wp, GPD)
            issue_w(0, "wg", 2, 2, wg, wp)
            issue_x(2, 2, GPD)
            issue_x(4, 4, GPD)
            issue_w(0, "wg", 4, 2, wg, wp)
            issue_w(0, "wg", 6, 2, wg, wp)
            issue_x(8, 4, GPD)
            issue_w(0, "wg", 8, 2, wg, wp)
            issue_w(0, "wg", 10, 2, wg, wp)
            issue_x(12, 4, GPD)
            for k0 in range(12, KO, KQ):
                issue_w(0, "wg", k0, KQ, wg, wp)
            for k0 in range(0, KO, KQ):
                issue_w(0, "wu", k0, KQ, wu, wp)
            for k0 in range(0, MO, DQ):
                issue_w(0, "wd", k0, DQ, wd, wdp)
            for e in range(1, E_PER):
                for k0 in range(0, KO, KQ):
                    issue_w(e, "wg", k0, KQ, wg, wp)
                for k0 in range(0, KO, KQ):
                    issue_w(e, "wu", k0, KQ, wu, wp)
                for k0 in range(0, MO, DQ):
                    issue_w(e, "wd", k0, DQ, wd, wdp)

            for e in range(E_PER):
                hT = hp.tile([P, MO, T], dt, tag="h")
                # all gate m-chunks first: the PE is gated only by the wg
                # stream, never by wu (which lands while these run).
                # Expert 0's gate runs k-outer so each wg tile is consumed
                # the moment its DMA lands (the stream is still ramping);
                # by expert 1 the weights are fully buffered and m-outer
                # staggers the silu/mult chain better.
                gss = []
                if e == 0:
                    pgs = [
                        ps.tile([P, T], f32, name=f"pg{m}", tag="ps")
                        for m in range(MO)
                    ]
                    for k in range(KO):
                        for m in range(MO):
                            nc.tensor.matmul(
                                pgs[m][:],
                                wslice(e, "wg", k, m * P, (m + 1) * P),
                                xslice(k),
                                start=(k == 0),
                                stop=(k == KO - 1),
                            )
                    for m in range(MO):
                        gs = gp.tile([P, T], f32, tag="g")
                        if sim_compat:
                            nc.scalar.activation(gs[:], pgs[m][:], AF.Sigmoid)
                            nc.vector.tensor_tensor(
                                gs[:], gs[:], pgs[m][:], mybir.AluOpType.mult
                            )
                        else:
                            nc.scalar.activation(gs[:], pgs[m][:], AF.Silu)
                        gss.append(gs)
                else:
                    for m in range(MO):
                        pg = ps.tile([P, T], f32, tag="ps")
                        for k in range(KO):
                            nc.tensor.matmul(
                                pg[:],
                                wslice(e, "wg", k, m * P, (m + 1) * P),
                                xslice(k),
                                start=(k == 0),
                                stop=(k == KO - 1),
                            )
                        gs = gp.tile([P, T], f32, tag="g")
                        if sim_compat:
                            nc.scalar.activation(gs[:], pg[:], AF.Sigmoid)
                            nc.vector.tensor_tensor(
                                gs[:], gs[:], pg[:], mybir.AluOpType.mult
                            )
                        else:
                            nc.scalar.activation(gs[:], pg[:], AF.Silu)
                        gss.append(gs)
                for m in range(MO):
                    pu = ps.tile([P, T], f32, tag="ps")
                    for k in range(KO):
                        nc.tensor.matmul(
                            pu[:],
                            wslice(e, "wu", k, m * P, (m + 1) * P),
                            xslice(k),
                            start=(k == 0),
                            stop=(k == KO - 1),
                        )
                    nc.vector.tensor_tensor(
                        hT[:, m, :], gss[m][:], pu[:], mybir.AluOpType.mult
                    )
                for mt in range(TO):
                    for n in range(NH):
                        po = ps.tile([P, NS], f32, tag="ps")
                        for k in range(MO):
                            nc.tensor.matmul(
                                po[:],
                                hT[:, k, mt * P : (mt + 1) * P],
                                wslice(e, "wd", k, n * NS, (n + 1) * NS),
                                start=(k == 0),
                                stop=(k == MO - 1),
                            )
                        # The very last tile is split in half so copy2
                        # overlaps dma1, shortening the end-of-kernel
                        # chain.
                        last = e == E_PER - 1 and mt == TO - 1 and n == NH - 1
                        if last:
                            oa = outp.tile([P, NS // 2], dt, tag="o")
                            nc.scalar.copy(oa[:], po[:, : NS // 2])
                            nc.sync.dma_start(
                                out[e, mt, :, n * NS : n * NS + NS // 2], oa[:]
                            )
                            ob = outp.tile([P, NS // 2], dt, tag="o")
                            nc.scalar.copy(ob[:], po[:, NS // 2 :])
                            nc.sync.dma_start(
                                out[e, mt, :, n * NS + NS // 2 : (n + 1) * NS], ob[:]
                            )
                        else:
                            ot = outp.tile([P, NS], dt, tag="o")
                            nc.scalar.copy(ot[:], po[:])
                            nc.sync.dma_start(
                                out[e, mt, :, n * NS : (n + 1) * NS], ot[:]
                            )

    nc.compile()
    return nc


def get_program(sim_compat=False):
    key = ("nc", sim_compat)
    if key not in _CACHE:
        _CACHE[key] = _build_program(sim_compat=sim_compat)
    return _CACHE[key]


def _prep_in_maps(hidden_states, w_gate, w_up, w_down):
    bf16 = ml_dtypes.bfloat16
    x = np.asarray(hidden_states, dtype=np.float32)
    wg = np.asarray(w_gate, dtype=np.float32)
    wu = np.asarray(w_up, dtype=np.float32)
    wd = np.asarray(w_down, dtype=np.float32)

    # xT: [H, T] -> [128, KO, T], partition p + chunk k <-> H index k*128+p
    xt = np.ascontiguousarray(
        x.T.reshape(KO, P, T).transpose(1, 0, 2).astype(bf16)
    )
    # w_gate/w_up: [E, I, H] -> per expert W.T = [H, I] -> [128, KO, I]
    wgt = np.ascontiguousarray(
        wg.transpose(0, 2, 1).reshape(E, KO, P, I).transpose(0, 2, 1, 3).astype(bf16)
    )
    wut = np.ascontiguousarray(
        wu.transpose(0, 2, 1).reshape(E, KO, P, I).transpose(0, 2, 1, 3).astype(bf16)
    )
    # w_down: [E, H, I] -> per expert W.T = [I, H] -> [128, MO, H]
    wdt = np.ascontiguousarray(
        wd.transpose(0, 2, 1).reshape(E, MO, P, H).transpose(0, 2, 1, 3).astype(bf16)
    )

    in_maps = []
    for c in range(N_CORES):
        sl = slice(c * E_PER, (c + 1) * E_PER)
        in_maps.append(
            {
                "xT": xt,
                "wg": np.ascontiguousarray(wgt[sl]),
                "wu": np.ascontiguousarray(wut[sl]),
                "wd": np.ascontiguousarray(wdt[sl]),
            }
        )
    return in_maps


def kernel(hidden_states, w_gate, w_up, w_down, _trace=False, _trace_kwargs=None):
    from concourse.bass_utils import run_bass_kernel_spmd

    nc = get_program()
    in_maps = _prep_in_maps(hidden_states, w_gate, w_up, w_down)
    kwargs = {}
    if _trace:
        kwargs = dict(trace=True, **(_trace_kwargs or {}))
    res = run_bass_kernel_spmd(nc, in_maps, core_ids=list(range(N_CORES)), **kwargs)
    out = np.concatenate(
        [
            res.results[c]["out"].reshape(E_PER * T, H).astype(np.float32)
            for c in range(N_CORES)
        ],
        axis=0,
    )
    if _trace:
        _CACHE["last_results"] = res
    return out



# revision 25
# speedup vs baseline: 1.0640x; 1.0590x over previous
"""DeepseekV3 MoE experts kernel for 8 Trainium2 NeuronCores.

Problem: every expert processes the FULL token set.
  g = x @ w_gate[e].T ; u = x @ w_up[e].T ; h = silu(g)*u
  out[e] = h @ w_down[e].T ;  concat over e -> [E*T, H]

Sharding: expert-parallel. Core c owns experts {2c, 2c+1}; hidden_states is
replicated; outputs are concatenated host-side (no on-device collectives).

Per-core compute (per expert e, with xT = x.T resident in SBUF):
  phase 1: gT[m*128:(m+1)*128, :] = wgT[:,k,mslice].T @ xT[:,k,:]  (acc over k)
  phase 2: same for uT; hT = silu(gT) * uT  (in [I, T] layout, no transposes)
  phase 3: out[mt*128.., nslice] = hT[:,k,mtslice].T @ wdT[:,k,nslice]

All matmul operands are bf16 (host-cast); PSUM accumulation is fp32 and the
output is written back in fp32. Weights stream in eighth-matrix tiles
(0.5 MiB DMAs) so the PE starts as soon as the first slice lands; the slot
pools back-pressure the DMA stream about one expert ahead.

Self-contained: shapes hardcoded; inputs are the full arrays from
setup_inputs(); returns the full [4096, 2048] fp32 output.
"""

import numpy as np
import ml_dtypes

E, T, H, I = 16, 256, 2048, 1024
N_CORES = 8
E_PER = E // N_CORES  # 2
P = 128
KO = H // P  # 16 k-chunks for phases 1/2
MO = I // P  # 8 m-chunks for phases 1/2 (= k-chunks for phase 3)
TO = T // P  # 2 m-chunks for phase 3
NS = 512  # n-slice width for phase 3
NH = H // NS  # 4

KQ = 2  # k-chunks per wg/wu eighth-tile (0.5 MiB DMAs: earlier PE start)
DQ = 2  # k-chunks per wd quarter-tile
XH = KO // 2  # k-chunks per x half-tile

_CACHE: dict = {}


def _build_program(sim_compat=False):
    # sim_compat: CoreSim lacks the Silu LUT — express silu as
    # sigmoid(g)*g with an extra DVE multiply. HW uses the fused Silu op.
    import concourse.mybir as mybir
    import concourse.tile as tile
    from concourse import bacc

    dt = mybir.dt.bfloat16
    f32 = mybir.dt.float32
    AF = mybir.ActivationFunctionType

    nc = bacc.Bacc(None, target_bir_lowering=False, debug=False)

    xT = nc.dram_tensor("xT", [P, KO, T], dt, kind="ExternalInput")[:]
    wg = nc.dram_tensor("wg", [E_PER, P, KO, I], dt, kind="ExternalInput")[:]
    wu = nc.dram_tensor("wu", [E_PER, P, KO, I], dt, kind="ExternalInput")[:]
    wd = nc.dram_tensor("wd", [E_PER, P, MO, H], dt, kind="ExternalInput")[:]
    # output in bf16: halves the output DMA bytes and the PSUM->SBUF copy
    # time; the host upcasts to fp32 (rounding adds ~1e-3 rel err, well
    # within the 2e-2 budget)
    out = nc.dram_tensor("out", [E_PER, TO, P, H], dt, kind="ExternalOutput")[:]

    with tile.TileContext(nc) as tc:
        with (
            tc.tile_pool(name="xp", bufs=6) as xp,
            tc.tile_pool(name="wp", bufs=24) as wp,
            tc.tile_pool(name="wdp", bufs=6) as wdp,
            tc.tile_pool(name="hp", bufs=2) as hp,
            tc.tile_pool(name="gp", bufs=8) as gp,
            tc.tile_pool(name="op", bufs=8) as outp,
            tc.tile_pool(name="ps", bufs=8, space="PSUM") as ps,
        ):
            # PE warm-up: matmuls on an uninitialized scratch tile with no
            # producer dependency, so they issue the moment the boot
            # barrier releases (~6.6us) and run while the first input DMAs
            # are still in flight. This flips the HAM clock gate to 8/8
            # before the first real matmul. 18 cold MMs ~= 4.4us of
            # sustained PE activity; the first real weight tiles land at
            # ~10.5us, right as the warm-up drains. (Values are garbage;
            # the PSUM tile is never read.)
            warm = xp.tile([P, T], dt, tag="warm")
            nc.vector.memset(warm[:], 0.0)
            wps = ps.tile([P, T], f32, tag="ps")
            for _ in range(18):
                nc.tensor.matmul(wps[:], warm[:, :P], warm[:], start=True, stop=True)

            xtiles = []  # (k0, kq, tile)
            wq: dict = {}  # (e, name) -> list of (k0, kq, tile)

            def issue_x(k0, kq, eng=None):
                t_ = xp.tile([P, kq, T], dt, tag="x")
                (eng or nc.sync).dma_start(t_[:], xT[:, k0 : k0 + kq, :])
                xtiles.append((k0, kq, t_))

            def xslice(k):
                for k0, kq, t_ in xtiles:
                    if k0 <= k < k0 + kq:
                        return t_[:, k - k0, :]
                raise KeyError(k)

            def issue_w(e, name, k0, kq, src, pool, eng=None):
                t_ = pool.tile([P, kq, src.shape[3]], dt, tag=pool.name)
                (eng or nc.sync).dma_start(t_[:], src[e, :, k0 : k0 + kq, :])
                wq.setdefault((e, name), []).append((k0, kq, t_))

            def wslice(e, name, k, lo, hi):
                for k0, kq, t_ in wq[(e, name)]:
                    if k0 <= k < k0 + kq:
                        return t_[:, k - k0, lo:hi]
                raise KeyError((e, name, k))

            # DMA issue order == consumption order. The leading slices are
            # extra small so the first matmul starts ~3us earlier; x is
            # woven into the first gate stream so the k-loop is never
            # input-starved. The slot pools back-pressure the stream.
            # All DMAs go through the SP HWDGE ring (nc.sync) — issuing
            # from the ACT ring measures consistently slower here. The
            # weave matches the gate k-outer consumption order: each x
            # chunk lands just before the wg chunks of the same k range.
            # All DMAs go through the SP HWDGE ring (nc.sync) in
            # consumption order — splitting the stream across the ACT
            # ring, the SWDGE (gpsimd) path, or into smaller kq=1
            # transfers all measured SLOWER; the DMA subsystem performs
            # best with one orderly FIFO of ~0.5 MiB transfers. The weave
            # matches the gate k-outer consumption order: each x chunk
            # lands just before the wg chunks of the same k range.
            issue_x(0, 2)
            issue_w(0, "wg", 0, 1, wg, wp)
            issue_w(0, "wg", 1, 1, wg, wp)
            issue_x(2, 2)
            issue_w(0, "wg", 2, 2, wg, wp)
            issue_x(4, 4)
            issue_w(0, "wg", 4, 2, wg, wp)
            issue_w(0, "wg", 6, 2, wg, wp)
            issue_x(8, 4)
            issue_w(0, "wg", 8, 2, wg, wp)
            issue_w(0, "wg", 10, 2, wg, wp)
            issue_x(12, 4)
            for k0 in range(12, KO, KQ):
                issue_w(0, "wg", k0, KQ, wg, wp)
            for k0 in range(0, KO, KQ):
                issue_w(0, "wu", k0, KQ, wu, wp)
            for k0 in range(0, MO, DQ):
                issue_w(0, "wd", k0, DQ, wd, wdp)
            for e in range(1, E_PER):
                for k0 in range(0, KO, KQ):
                    issue_w(e, "wg", k0, KQ, wg, wp)
                for k0 in range(0, KO, KQ):
                    issue_w(e, "wu", k0, KQ, wu, wp)
                for k0 in range(0, MO, DQ):
                    issue_w(e, "wd", k0, DQ, wd, wdp)

            for e in range(E_PER):
                hT = hp.tile([P, MO, T], dt, tag="h")
                # all gate m-chunks first: the PE is gated only by the wg
                # stream, never by wu (which lands while these run).
                # Expert 0's gate runs k-outer so each wg tile is consumed
                # the moment its DMA lands (the stream is still ramping);
                # by expert 1 the weights are fully buffered and m-outer
                # staggers the silu/mult chain better.
                gss = []
                if e == 0:
                    pgs = [
                        ps.tile([P, T], f32, name=f"pg{m}", tag="ps")
                        for m in range(MO)
                    ]
                    for k in range(KO):
                        for m in range(MO):
                            nc.tensor.matmul(
                                pgs[m][:],
                                wslice(e, "wg", k, m * P, (m + 1) * P),
                                xslice(k),
                                start=(k == 0),
                                stop=(k == KO - 1),
                            )
                    for m in range(MO):
                        gs = gp.tile([P, T], f32, tag="g")
                        if sim_compat:
                            nc.scalar.activation(gs[:], pgs[m][:], AF.Sigmoid)
                            nc.vector.tensor_tensor(
                                gs[:], gs[:], pgs[m][:], mybir.AluOpType.mult
                            )
                        else:
                            nc.scalar.activation(gs[:], pgs[m][:], AF.Silu)
                        gss.append(gs)
                else:
                    for m in range(MO):
                        pg = ps.tile([P, T], f32, tag="ps")
                        for k in range(KO):
                            nc.tensor.matmul(
                                pg[:],
                                wslice(e, "wg", k, m * P, (m + 1) * P),
                                xslice(k),
                                start=(k == 0),
                                stop=(k == KO - 1),
                            )
                        gs = gp.tile([P, T], f32, tag="g")
                        if sim_compat:
                            nc.scalar.activation(gs[:], pg[:], AF.Sigmoid)
                            nc.vector.tensor_tensor(
                                gs[:], gs[:], pg[:], mybir.AluOpType.mult
                            )
                        else:
                            nc.scalar.activation(gs[:], pg[:], AF.Silu)
                        gss.append(gs)
                for m in range(MO):
                    pu = ps.tile([P, T], f32, tag="ps")
                    for k in range(KO):
                        nc.tensor.matmul(
                            pu[:],
                            wslice(e, "wu", k, m * P, (m + 1) * P),
                            xslice(k),
                            start=(k == 0),
                            stop=(k == KO - 1),
                        )
                    nc.vector.tensor_tensor(
                        hT[:, m, :], gss[m][:], pu[:], mybir.AluOpType.mult
                    )
                for mt in range(TO):
                    for n in range(NH):
                        # The very last tile runs as two half-width
                        # accumulations so the first half's copy+DMA
                        # overlaps the second half's matmuls, shortening
                        # the end-of-kernel chain.
                        last = e == E_PER - 1 and mt == TO - 1 and n == NH - 1
                        if last:
                            for h2 in range(2):
                                lo = n * NS + h2 * (NS // 2)
                                po = ps.tile([P, NS // 2], f32, tag="ps")
                                for k in range(MO):
                                    nc.tensor.matmul(
                                        po[:],
                                        hT[:, k, mt * P : (mt + 1) * P],
                                        wslice(e, "wd", k, lo, lo + NS // 2),
                                        start=(k == 0),
                                        stop=(k == MO - 1),
                                    )
                                ot = outp.tile([P, NS // 2], dt, tag="o")
                                nc.scalar.copy(ot[:], po[:])
                                nc.sync.dma_start(
                                    out[e, mt, :, lo : lo + NS // 2], ot[:]
                                )
                        else:
                            po = ps.tile([P, NS], f32, tag="ps")
                            for k in range(MO):
                                nc.tensor.matmul(
                                    po[:],
                                    hT[:, k, mt * P : (mt + 1) * P],
                                    wslice(e, "wd", k, n * NS, (n + 1) * NS),
                                    start=(k == 0),
                                    stop=(k == MO - 1),
                                )
                            ot = outp.tile([P, NS], dt, tag="o")
                            nc.scalar.copy(ot[:], po[:])
                            nc.sync.dma_start(
                                out[e, mt, :, n * NS : (n + 1) * NS], ot[:]
                            )

    nc.compile()
    return nc


def get_program(sim_compat=False):
    key = ("nc", sim_compat)
    if key not in _CACHE:
        _CACHE[key] = _build_program(sim_compat=sim_compat)
    return _CACHE[key]


def _prep_in_maps(hidden_states, w_gate, w_up, w_down):
    bf16 = ml_dtypes.bfloat16
    x = np.asarray(hidden_states, dtype=np.float32)
    wg = np.asarray(w_gate, dtype=np.float32)
    wu = np.asarray(w_up, dtype=np.float32)
    wd = np.asarray(w_down, dtype=np.float32)

    # xT: [H, T] -> [128, KO, T], partition p + chunk k <-> H index k*128+p
    xt = np.ascontiguousarray(
        x.T.reshape(KO, P, T).transpose(1, 0, 2).astype(bf16)
    )
    # w_gate/w_up: [E, I, H] -> per expert W.T = [H, I] -> [128, KO, I]
    wgt = np.ascontiguousarray(
        wg.transpose(0, 2, 1).reshape(E, KO, P, I).transpose(0, 2, 1, 3).astype(bf16)
    )
    wut = np.ascontiguousarray(
        wu.transpose(0, 2, 1).reshape(E, KO, P, I).transpose(0, 2, 1, 3).astype(bf16)
    )
    # w_down: [E, H, I] -> per expert W.T = [I, H] -> [128, MO, H]
    wdt = np.ascontiguousarray(
        wd.transpose(0, 2, 1).reshape(E, MO, P, H).transpose(0, 2, 1, 3).astype(bf16)
    )

    in_maps = []
    for c in range(N_CORES):
        sl = slice(c * E_PER, (c + 1) * E_PER)
        in_maps.append(
            {
                "xT": xt,
                "wg": np.ascontiguousarray(wgt[sl]),
                "wu": np.ascontiguousarray(wut[sl]),
                "wd": np.ascontiguousarray(wdt[sl]),
            }
        )
    return in_maps


def kernel(hidden_states, w_gate, w_up, w_down, _trace=False, _trace_kwargs=None):
    from concourse.bass_utils import run_bass_kernel_spmd

    nc = get_program()
    in_maps = _prep_in_maps(hidden_states, w_gate, w_up, w_down)
    kwargs = {}
    if _trace:
        kwargs = dict(trace=True, **(_trace_kwargs or {}))
    res = run_bass_kernel_spmd(nc, in_maps, core_ids=list(range(N_CORES)), **kwargs)
    out = np.concatenate(
        [
            res.results[c]["out"].reshape(E_PER * T, H).astype(np.float32)
            for c in range(N_CORES)
        ],
        axis=0,
    )
    if _trace:
        _CACHE["last_results"] = res
    return out



# revision 26
# speedup vs baseline: 1.0818x; 1.0167x over previous
"""DeepseekV3 MoE experts kernel for 8 Trainium2 NeuronCores.

Problem: every expert processes the FULL token set.
  g = x @ w_gate[e].T ; u = x @ w_up[e].T ; h = silu(g)*u
  out[e] = h @ w_down[e].T ;  concat over e -> [E*T, H]

Sharding: expert-parallel. Core c owns experts {2c, 2c+1}; hidden_states is
replicated; outputs are concatenated host-side (no on-device collectives).

Per-core compute (per expert e, with xT = x.T resident in SBUF):
  phase 1: gT[m*128:(m+1)*128, :] = wgT[:,k,mslice].T @ xT[:,k,:]  (acc over k)
  phase 2: same for uT; hT = silu(gT) * uT  (in [I, T] layout, no transposes)
  phase 3: out[mt*128.., nslice] = hT[:,k,mtslice].T @ wdT[:,k,nslice]

All matmul operands are bf16 (host-cast); PSUM accumulation is fp32 and the
output is written back in fp32. Weights stream in eighth-matrix tiles
(0.5 MiB DMAs) so the PE starts as soon as the first slice lands; the slot
pools back-pressure the DMA stream about one expert ahead.

Self-contained: shapes hardcoded; inputs are the full arrays from
setup_inputs(); returns the full [4096, 2048] fp32 output.
"""

import numpy as np
import ml_dtypes

E, T, H, I = 16, 256, 2048, 1024
N_CORES = 8
E_PER = E // N_CORES  # 2
P = 128
KO = H // P  # 16 k-chunks for phases 1/2
MO = I // P  # 8 m-chunks for phases 1/2 (= k-chunks for phase 3)
TO = T // P  # 2 m-chunks for phase 3
NS = 512  # n-slice width for phase 3
NH = H // NS  # 4

KQ = 2  # k-chunks per wg/wu eighth-tile (0.5 MiB DMAs: earlier PE start)
DQ = 2  # k-chunks per wd quarter-tile
XH = KO // 2  # k-chunks per x half-tile

_CACHE: dict = {}


def _build_program(sim_compat=False):
    # sim_compat: CoreSim lacks the Silu LUT — express silu as
    # sigmoid(g)*g with an extra DVE multiply. HW uses the fused Silu op.
    import concourse.mybir as mybir
    import concourse.tile as tile
    from concourse import bacc

    dt = mybir.dt.bfloat16
    f32 = mybir.dt.float32
    AF = mybir.ActivationFunctionType

    nc = bacc.Bacc(None, target_bir_lowering=False, debug=False)

    xT = nc.dram_tensor("xT", [P, KO, T], dt, kind="ExternalInput")[:]
    wg = nc.dram_tensor("wg", [E_PER, P, KO, I], dt, kind="ExternalInput")[:]
    wu = nc.dram_tensor("wu", [E_PER, P, KO, I], dt, kind="ExternalInput")[:]
    wd = nc.dram_tensor("wd", [E_PER, P, MO, H], dt, kind="ExternalInput")[:]
    # output in bf16: halves the output DMA bytes and the PSUM->SBUF copy
    # time; the host upcasts to fp32 (rounding adds ~1e-3 rel err, well
    # within the 2e-2 budget)
    out = nc.dram_tensor("out", [E_PER, TO, P, H], dt, kind="ExternalOutput")[:]

    with tile.TileContext(nc) as tc:
        with (
            tc.tile_pool(name="xp", bufs=6) as xp,
            tc.tile_pool(name="wp", bufs=26) as wp,
            tc.tile_pool(name="wdp", bufs=6) as wdp,
            tc.tile_pool(name="hp", bufs=2) as hp,
            tc.tile_pool(name="gp", bufs=8) as gp,
            tc.tile_pool(name="op", bufs=8) as outp,
            tc.tile_pool(name="ps", bufs=8, space="PSUM") as ps,
        ):
            # PE warm-up: matmuls on an uninitialized scratch tile with no
            # producer dependency, so they issue the moment the boot
            # barrier releases (~6.6us) and run while the first input DMAs
            # are still in flight. This flips the HAM clock gate to 8/8
            # before the first real matmul. 18 cold MMs ~= 4.4us of
            # sustained PE activity; the first real weight tiles land at
            # ~10.5us, right as the warm-up drains. (Values are garbage;
            # the PSUM tile is never read.)
            warm = xp.tile([P, T], dt, tag="warm")
            nc.vector.memset(warm[:], 0.0)
            wps = ps.tile([P, T], f32, tag="ps")
            for _ in range(18):
                nc.tensor.matmul(wps[:], warm[:, :P], warm[:], start=True, stop=True)

            xtiles = []  # (k0, kq, tile)
            wq: dict = {}  # (e, name) -> list of (k0, kq, tile)

            def issue_x(k0, kq, eng=None):
                t_ = xp.tile([P, kq, T], dt, tag="x")
                (eng or nc.sync).dma_start(t_[:], xT[:, k0 : k0 + kq, :])
                xtiles.append((k0, kq, t_))

            def xslice(k):
                for k0, kq, t_ in xtiles:
                    if k0 <= k < k0 + kq:
                        return t_[:, k - k0, :]
                raise KeyError(k)

            def issue_w(e, name, k0, kq, src, pool, eng=None):
                t_ = pool.tile([P, kq, src.shape[3]], dt, tag=pool.name)
                (eng or nc.sync).dma_start(t_[:], src[e, :, k0 : k0 + kq, :])
                wq.setdefault((e, name), []).append((k0, kq, t_))

            def wslice(e, name, k, lo, hi):
                for k0, kq, t_ in wq[(e, name)]:
                    if k0 <= k < k0 + kq:
                        return t_[:, k - k0, lo:hi]
                raise KeyError((e, name, k))

            # DMA issue order == consumption order. The leading slices are
            # extra small so the first matmul starts ~3us earlier; x is
            # woven into the first gate stream so the k-loop is never
            # input-starved. The slot pools back-pressure the stream.
            # All DMAs go through the SP HWDGE ring (nc.sync) — issuing
            # from the ACT ring measures consistently slower here. The
            # weave matches the gate k-outer consumption order: each x
            # chunk lands just before the wg chunks of the same k range.
            # All DMAs go through the SP HWDGE ring (nc.sync) in
            # consumption order — splitting the stream across the ACT
            # ring, the SWDGE (gpsimd) path, or into smaller kq=1
            # transfers all measured SLOWER; the DMA subsystem performs
            # best with one orderly FIFO of ~0.5 MiB transfers. The weave
            # matches the gate k-outer consumption order: each x chunk
            # lands just before the wg chunks of the same k range.
            issue_x(0, 2)
            issue_w(0, "wg", 0, 1, wg, wp)
            issue_w(0, "wg", 1, 1, wg, wp)
            issue_x(2, 2)
            issue_w(0, "wg", 2, 2, wg, wp)
            issue_x(4, 4)
            issue_w(0, "wg", 4, 2, wg, wp)
            issue_w(0, "wg", 6, 2, wg, wp)
            issue_x(8, 4)
            issue_w(0, "wg", 8, 2, wg, wp)
            issue_w(0, "wg", 10, 2, wg, wp)
            issue_x(12, 4)
            for k0 in range(12, KO, KQ):
                issue_w(0, "wg", k0, KQ, wg, wp)
            for k0 in range(0, KO, KQ):
                issue_w(0, "wu", k0, KQ, wu, wp)
            for k0 in range(0, MO, DQ):
                issue_w(0, "wd", k0, DQ, wd, wdp)
            for e in range(1, E_PER):
                for k0 in range(0, KO, KQ):
                    issue_w(e, "wg", k0, KQ, wg, wp)
                for k0 in range(0, KO, KQ):
                    issue_w(e, "wu", k0, KQ, wu, wp)
                for k0 in range(0, MO, DQ):
                    issue_w(e, "wd", k0, DQ, wd, wdp)

            for e in range(E_PER):
                hT = hp.tile([P, MO, T], dt, tag="h")
                # all gate m-chunks first: the PE is gated only by the wg
                # stream, never by wu (which lands while these run).
                # Expert 0's gate runs k-outer so each wg tile is consumed
                # the moment its DMA lands (the stream is still ramping);
                # by expert 1 the weights are fully buffered and m-outer
                # staggers the silu/mult chain better.
                gss = []
                if e == 0:
                    pgs = [
                        ps.tile([P, T], f32, name=f"pg{m}", tag="ps")
                        for m in range(MO)
                    ]
                    for k in range(KO):
                        for m in range(MO):
                            nc.tensor.matmul(
                                pgs[m][:],
                                wslice(e, "wg", k, m * P, (m + 1) * P),
                                xslice(k),
                                start=(k == 0),
                                stop=(k == KO - 1),
                            )
                    for m in range(MO):
                        gs = gp.tile([P, T], f32, tag="g")
                        if sim_compat:
                            nc.scalar.activation(gs[:], pgs[m][:], AF.Sigmoid)
                            nc.vector.tensor_tensor(
                                gs[:], gs[:], pgs[m][:], mybir.AluOpType.mult
                            )
                        else:
                            nc.scalar.activation(gs[:], pgs[m][:], AF.Silu)
                        gss.append(gs)
                else:
                    for m in range(MO):
                        pg = ps.tile([P, T], f32, tag="ps")
                        for k in range(KO):
                            nc.tensor.matmul(
                                pg[:],
                                wslice(e, "wg", k, m * P, (m + 1) * P),
                                xslice(k),
                                start=(k == 0),
                                stop=(k == KO - 1),
                            )
                        gs = gp.tile([P, T], f32, tag="g")
                        if sim_compat:
                            nc.scalar.activation(gs[:], pg[:], AF.Sigmoid)
                            nc.vector.tensor_tensor(
                                gs[:], gs[:], pg[:], mybir.AluOpType.mult
                            )
                        else:
                            nc.scalar.activation(gs[:], pg[:], AF.Silu)
                        gss.append(gs)
                for m in range(MO):
                    pu = ps.tile([P, T], f32, tag="ps")
                    for k in range(KO):
                        nc.tensor.matmul(
                            pu[:],
                            wslice(e, "wu", k, m * P, (m + 1) * P),
                            xslice(k),
                            start=(k == 0),
                            stop=(k == KO - 1),
                        )
                    nc.vector.tensor_tensor(
                        hT[:, m, :], gss[m][:], pu[:], mybir.AluOpType.mult
                    )
                for mt in range(TO):
                    for n in range(NH):
                        # The very last tile runs as two half-width
                        # accumulations so the first half's copy+DMA
                        # overlaps the second half's matmuls, shortening
                        # the end-of-kernel chain.
                        last = e == E_PER - 1 and mt == TO - 1 and n == NH - 1
                        if last:
                            for h2 in range(2):
                                lo = n * NS + h2 * (NS // 2)
                                po = ps.tile([P, NS // 2], f32, tag="ps")
                                for k in range(MO):
                                    nc.tensor.matmul(
                                        po[:],
                                        hT[:, k, mt * P : (mt + 1) * P],
                                        wslice(e, "wd", k, lo, lo + NS // 2),
                                        start=(k == 0),
                                        stop=(k == MO - 1),
                                    )
                                ot = outp.tile([P, NS // 2], dt, tag="o")
                                nc.scalar.copy(ot[:], po[:])
                                nc.sync.dma_start(
                                    out[e, mt, :, lo : lo + NS // 2], ot[:]
                                )
                        else:
                            po = ps.tile([P, NS], f32, tag="ps")
                            for k in range(MO):
                                nc.tensor.matmul(
                                    po[:],
                                    hT[:, k, mt * P : (mt + 1) * P],
                                    wslice(e, "wd", k, n * NS, (n + 1) * NS),
                                    start=(k == 0),
                                    stop=(k == MO - 1),
                                )
                            ot = outp.tile([P, NS], dt, tag="o")
                            nc.scalar.copy(ot[:], po[:])
                            nc.sync.dma_start(
                                out[e, mt, :, n * NS : (n + 1) * NS], ot[:]
                            )

    nc.compile()
    return nc


def get_program(sim_compat=False):
    key = ("nc", sim_compat)
    if key not in _CACHE:
        _CACHE[key] = _build_program(sim_compat=sim_compat)
    return _CACHE[key]


def _prep_in_maps(hidden_states, w_gate, w_up, w_down):
    bf16 = ml_dtypes.bfloat16
    x = np.asarray(hidden_states, dtype=np.float32)
    wg = np.asarray(w_gate, dtype=np.float32)
    wu = np.asarray(w_up, dtype=np.float32)
    wd = np.asarray(w_down, dtype=np.float32)

    # xT: [H, T] -> [128, KO, T], partition p + chunk k <-> H index k*128+p
    xt = np.ascontiguousarray(
        x.T.reshape(KO, P, T).transpose(1, 0, 2).astype(bf16)
    )
    # w_gate/w_up: [E, I, H] -> per expert W.T = [H, I] -> [128, KO, I]
    wgt = np.ascontiguousarray(
        wg.transpose(0, 2, 1).reshape(E, KO, P, I).transpose(0, 2, 1, 3).astype(bf16)
    )
    wut = np.ascontiguousarray(
        wu.transpose(0, 2, 1).reshape(E, KO, P, I).transpose(0, 2, 1, 3).astype(bf16)
    )
    # w_down: [E, H, I] -> per expert W.T = [I, H] -> [128, MO, H]
    wdt = np.ascontiguousarray(
        wd.transpose(0, 2, 1).reshape(E, MO, P, H).transpose(0, 2, 1, 3).astype(bf16)
    )

    in_maps = []
    for c in range(N_CORES):
        sl = slice(c * E_PER, (c + 1) * E_PER)
        in_maps.append(
            {
                "xT": xt,
                "wg": np.ascontiguousarray(wgt[sl]),
                "wu": np.ascontiguousarray(wut[sl]),
                "wd": np.ascontiguousarray(wdt[sl]),
            }
        )
    return in_maps


def kernel(hidden_states, w_gate, w_up, w_down, _trace=False, _trace_kwargs=None):
    from concourse.bass_utils import run_bass_kernel_spmd

    nc = get_program()
    in_maps = _prep_in_maps(hidden_states, w_gate, w_up, w_down)
    kwargs = {}
    if _trace:
        kwargs = dict(trace=True, **(_trace_kwargs or {}))
    res = run_bass_kernel_spmd(nc, in_maps, core_ids=list(range(N_CORES)), **kwargs)
    out = np.concatenate(
        [
            res.results[c]["out"].reshape(E_PER * T, H).astype(np.float32)
            for c in range(N_CORES)
        ],
        axis=0,
    )
    if _trace:
        _CACHE["last_results"] = res
    return out

